# revision 4
# baseline (speedup 1.0000x reference)
"""Multi-head attention Trainium2 kernel (8 NeuronCores, SPMD).

Sharding: core c handles batch b = c//2 and query-token half c%2 (1024 of
2048 rows). Each core redundantly computes the K/V projections for its
batch (cores sharing a batch compute the same k/v) -- no collectives.

All matmuls run in float32r (TF32-like precision, ~4x faster than fp32 on
the PE). Layout strategy:
  - host pre-transposes Q/K/V slices and weights so the contraction dim is
    always on SBUF partitions
  - k/q projections produce transposed outputs [out_feat, tok]
  - v projection produces natural [tok, feat]; k^T and v round-trip through
    DRAM and are re-streamed per head-pair during attention
  - scores computed transposed [k_tok, q_tok]; softmax skips the max
    subtraction (|scores| < ~40 guaranteed by input scale); exp on ACT
  - attn @ v computed as v^T @ attn with a ones-column in v producing the
    softmax denominators for free (M=65)
  - normalization: reciprocal of sums broadcast across partitions via a
    K=1 matmul, then a vector multiply
  - o_proj consumes the accumulated [feat, tok] attention output and writes
    the natural-layout result
"""

import numpy as np

import concourse.bass as bass
import concourse.mybir as mybir
import concourse.tile as tile
from concourse import bacc
from concourse.bass_utils import run_bass_kernel_spmd

F32 = mybir.dt.float32
F32R = mybir.dt.float32r
AF = mybir.ActivationFunctionType

H, DM, DK = 16, 1024, 64
B, L = 4, 2048
TOK = 1024          # query tokens per core
FT = DM // 128      # 8 feature tiles
NCORES = 8
SCALE = 1.0 / np.sqrt(DK)

_cache = {}


def _build_nc():
    nc = bacc.Bacc(None, target_bir_lowering=False)

    qt = nc.dram_tensor("qt", [DM, TOK], F32R, kind="ExternalInput")
    kt = nc.dram_tensor("kt", [DM, L], F32R, kind="ExternalInput")
    vt = nc.dram_tensor("vt", [DM, L], F32R, kind="ExternalInput")
    wkt = nc.dram_tensor("wkt", [DM, DM], F32R, kind="ExternalInput")
    wvt = nc.dram_tensor("wvt", [DM, DM], F32R, kind="ExternalInput")
    wqt = nc.dram_tensor("wqt", [DM, DM], F32R, kind="ExternalInput")
    wot = nc.dram_tensor("wot", [DM, DM], F32R, kind="ExternalInput")
    bk_d = nc.dram_tensor("bk", [DM], F32, kind="ExternalInput")
    bq_d = nc.dram_tensor("bq", [DM], F32, kind="ExternalInput")
    bv_d = nc.dram_tensor("bv", [DM], F32, kind="ExternalInput")
    bo_d = nc.dram_tensor("bo", [DM], F32, kind="ExternalInput")
    out_d = nc.dram_tensor("out", [TOK, DM], F32, kind="ExternalOutput")

    def bcast_ap(dram_ap, parts=128):
        return bass.AP(
            tensor=dram_ap.tensor,
            offset=dram_ap.offset,
            ap=[[0, parts]] + [list(d) for d in dram_ap.ap],
        )

    from contextlib import ExitStack

    with tile.TileContext(nc) as tc, ExitStack() as ctx:
        dram = ctx.enter_context(tc.tile_pool(name="dram", bufs=1, space="DRAM"))
        kT_dram = dram.tile([DM, L], F32R, name="kT_dram")
        v_dram = dram.tile([L, DM], F32R, name="v_dram")

        persist = ctx.enter_context(tc.tile_pool(name="persist", bufs=1))
        bk_sb = persist.tile([128, FT], F32, name="bk_sb")
        nc.sync.dma_start(bk_sb[:], bk_d[:].rearrange("(f p) -> p f", p=128))
        bq_sb = persist.tile([128, FT], F32, name="bq_sb")
        nc.sync.dma_start(bq_sb[:], bq_d[:].rearrange("(f p) -> p f", p=128))
        bv_rep = persist.tile([128, DM], F32, name="bv_rep")
        nc.gpsimd.dma_start(bv_rep[:], bcast_ap(bv_d[:]))
        bo_rep = persist.tile([128, DM], F32, name="bo_rep")
        nc.gpsimd.dma_start(bo_rep[:], bcast_ap(bo_d[:]))
        ones_f32 = persist.tile([65, DK], F32, name="ones_f32")
        nc.vector.memset(ones_f32[:], 1.0)
        ones_t = persist.tile([65, DK], F32R, name="ones_t")
        nc.vector.tensor_copy(ones_t[:], ones_f32[:])
        ones_col = persist.tile([128, 16, 2, 1], F32, name="ones_col")
        nc.vector.memset(ones_col[:], 1.0)

        # ---- Phase 1: k-proj (transposed out) and v-proj (natural out) ----
        with (
            tc.tile_pool(name="p1w", bufs=1) as wpool,
            tc.tile_pool(name="p1c", bufs=2) as cpool,
            tc.tile_pool(name="p1s", bufs=3) as spool,
            tc.tile_pool(name="psA", bufs=2, space="PSUM") as psA,
        ):
            wk_sb = wpool.tile([128, FT, DM], F32R, name="wk_sb")
            nc.sync.dma_start(wk_sb[:], wkt[:, :].rearrange("(f p) o -> p f o", p=128))
            wv_sb = wpool.tile([128, FT, DM], F32R, name="wv_sb")
            nc.sync.dma_start(wv_sb[:], wvt[:, :].rearrange("(f p) o -> p f o", p=128))

            # k^T = (Wk^T)^T @ K^T : lhsT = WkT tile, moving = K^T chunk
            for n in range(4):
                ktc = cpool.tile([128, FT, 512], F32R, name="ktc")
                nc.sync.dma_start(
                    ktc[:],
                    kt[:, n * 512:(n + 1) * 512].rearrange("(f p) t -> p f t", p=128),
                )
                for m in range(FT):
                    ps = psA.tile([128, 512], F32, name="psA")
                    for k in range(FT):
                        nc.tensor.matmul(
                            ps[:],
                            wk_sb[:, k, m * 128:(m + 1) * 128],
                            ktc[:, k, :],
                            start=(k == 0),
                            stop=(k == FT - 1),
                        )
                    stg = spool.tile([128, 512], F32R, name="stg")
                    nc.scalar.activation(stg[:], ps[:], AF.Identity, bias=bk_sb[:, m:m + 1])
                    nc.sync.dma_start(
                        kT_dram[m * 128:(m + 1) * 128, n * 512:(n + 1) * 512], stg[:]
                    )

            # v = (V^T)^T @ Wv^T : lhsT = V^T tile, moving = WvT chunk
            for m in range(16):
                vtc = cpool.tile([128, FT, 128], F32R, name="vtc")
                nc.sync.dma_start(
                    vtc[:],
                    vt[:, m * 128:(m + 1) * 128].rearrange("(f p) t -> p f t", p=128),
                )
                for n in range(2):
                    ps = psA.tile([128, 512], F32, name="psA")
                    for k in range(FT):
                        nc.tensor.matmul(
                            ps[:],
                            vtc[:, k, :],
                            wv_sb[:, k, n * 512:(n + 1) * 512],
                            start=(k == 0),
                            stop=(k == FT - 1),
                        )
                    stg = spool.tile([128, 512], F32R, name="stgv")
                    nc.vector.tensor_add(stg[:], ps[:], bv_rep[:, n * 512:(n + 1) * 512])
                    nc.sync.dma_start(
                        v_dram[m * 128:(m + 1) * 128, n * 512:(n + 1) * 512], stg[:]
                    )

        # ---- Phase 2: q-proj (transposed out, SBUF resident) ----
        qTpool = ctx.enter_context(tc.tile_pool(name="qTp", bufs=1))
        qT_sb = qTpool.tile([128, FT, TOK], F32R, name="qT_sb")
        with (
            tc.tile_pool(name="p3w", bufs=1) as wq_pool,
            tc.tile_pool(name="p3c", bufs=2) as qc_pool,
            tc.tile_pool(name="psB", bufs=2, space="PSUM") as psB,
        ):
            wq_sb = wq_pool.tile([128, FT, DM], F32R, name="wq_sb")
            nc.sync.dma_start(wq_sb[:], wqt[:, :].rearrange("(f p) o -> p f o", p=128))
            for n in range(2):
                qtc = qc_pool.tile([128, FT, 512], F32R, name="qtc")
                nc.sync.dma_start(
                    qtc[:],
                    qt[:, n * 512:(n + 1) * 512].rearrange("(f p) t -> p f t", p=128),
                )
                for m in range(FT):
                    ps = psB.tile([128, 512], F32, name="psB")
                    for k in range(FT):
                        nc.tensor.matmul(
                            ps[:],
                            wq_sb[:, k, m * 128:(m + 1) * 128],
                            qtc[:, k, :],
                            start=(k == 0),
                            stop=(k == FT - 1),
                        )
                    nc.scalar.activation(
                        qT_sb[:, m, n * 512:(n + 1) * 512],
                        ps[:],
                        AF.Identity,
                        bias=bq_sb[:, m:m + 1],
                    )

        # ---- Phase 3: attention, head pairs ----
        aout_pool = ctx.enter_context(tc.tile_pool(name="aout", bufs=1))
        attn_outT = aout_pool.tile([128, FT, TOK], F32R, name="attn_outT")
        with (
            tc.tile_pool(name="kv", bufs=2) as kv_pool,
            tc.tile_pool(name="at", bufs=4) as at_pool,
            tc.tile_pool(name="nrm", bufs=2) as nrm_pool,
            tc.tile_pool(name="ps_sc", bufs=3, space="PSUM") as ps_sc,
            tc.tile_pool(name="ps_av", bufs=1, space="PSUM") as ps_av,
        ):
            v_dram_r = v_dram[:, :].rearrange("(kt p) (h d) -> p kt h d", p=128, d=DK)
            for pair in range(8):
                kT_pair = kv_pool.tile([128, L], F32R, name="kT_pair")
                nc.sync.dma_start(kT_pair[:], kT_dram[pair * 128:(pair + 1) * 128, :])
                v_pair = kv_pool.tile([128, 16, 2, DK + 1], F32R, name="v_pair")
                for hh in range(2):
                    nc.sync.dma_start(
                        v_pair[:, :, hh, 0:DK],
                        v_dram_r[:, :, 2 * pair + hh, :],
                    )
                nc.vector.tensor_copy(v_pair[:, :, :, DK:DK + 1], ones_col[:])
                for hh in range(2):
                    base = hh * 64
                    av = ps_av.tile([65, TOK], F32, name="av")
                    for kt_i in range(16):
                        sc = ps_sc.tile([128, TOK], F32, name="sc", tag="sc")
                        for qh in range(2):
                            nc.tensor.matmul(
                                sc[:, qh * 512:(qh + 1) * 512],
                                kT_pair[base:base + 64, kt_i * 128:(kt_i + 1) * 128],
                                qT_sb[base:base + 64, pair, qh * 512:(qh + 1) * 512],
                                start=True,
                                stop=True,
                            )
                        atn = at_pool.tile([128, TOK], F32R, name="atn")
                        nc.scalar.activation(atn[:], sc[:], AF.Exp, scale=SCALE)
                        for qh in range(2):
                            nc.tensor.matmul(
                                av[:, qh * 512:(qh + 1) * 512],
                                v_pair[:, kt_i, hh, :],
                                atn[:, qh * 512:(qh + 1) * 512],
                                start=(kt_i == 0),
                                stop=(kt_i == 15),
                            )
                    # normalization: out_h = av[0:64] / av[64]
                    recip = nrm_pool.tile([65, TOK], F32R, name="recip")
                    with nc.allow_low_precision(reason="softmax denom reciprocal"):
                        nc.vector.reciprocal(recip[64:65, :], av[64:65, :])
                    bc = ps_sc.tile([64, TOK], F32, name="bc", tag="sc")
                    for qh in range(2):
                        nc.tensor.matmul(
                            bc[:, qh * 512:(qh + 1) * 512],
                            ones_t[64:65, :],
                            recip[64:65, qh * 512:(qh + 1) * 512],
                            start=True,
                            stop=True,
                        )
                    bc_sb = nrm_pool.tile([64, TOK], F32, name="bc_sb")
                    nc.vector.tensor_copy(bc_sb[:], bc[:])
                    if hh == 0:
                        nc.vector.tensor_mul(
                            attn_outT[0:64, pair, :], av[0:64, :], bc_sb[:]
                        )
                    else:
                        tmp = nrm_pool.tile([64, TOK], F32R, name="tmp")
                        nc.vector.tensor_mul(tmp[:], av[0:64, :], bc_sb[:])
                        nc.sync.dma_start(attn_outT[64:128, pair, :], tmp[:])

        # ---- Phase 4: o-proj ----
        with (
            tc.tile_pool(name="p5w", bufs=2) as wo_pool,
            tc.tile_pool(name="p5o", bufs=3) as o_pool,
            tc.tile_pool(name="psC", bufs=2, space="PSUM") as psC,
        ):
            for n in range(2):
                wo_c = wo_pool.tile([128, FT, 512], F32R, name="wo_c")
                nc.sync.dma_start(
                    wo_c[:],
                    wot[:, n * 512:(n + 1) * 512].rearrange("(f p) o -> p f o", p=128),
                )
                for m in range(FT):
                    ps = psC.tile([128, 512], F32, name="psC")
                    for k in range(FT):
                        nc.tensor.matmul(
                            ps[:],
                            attn_outT[:, k, m * 128:(m + 1) * 128],
                            wo_c[:, k, :],
                            start=(k == 0),
                            stop=(k == FT - 1),
                        )
                    osb = o_pool.tile([128, 512], F32, name="osb")
                    nc.vector.tensor_add(osb[:], ps[:], bo_rep[:, n * 512:(n + 1) * 512])
                    nc.sync.dma_start(
                        out_d[m * 128:(m + 1) * 128, n * 512:(n + 1) * 512], osb[:]
                    )

    nc.compile()
    return nc


def _get_nc():
    if "nc" not in _cache:
        _cache["nc"] = _build_nc()
    return _cache["nc"]


def _make_in_maps(V, K, Q, Wv, bv, Wk, bk, Wq, bq, Wo, bo):
    f32 = np.float32
    WqT = np.ascontiguousarray(Wq.T, dtype=f32)
    WkT = np.ascontiguousarray(Wk.T, dtype=f32)
    WvT = np.ascontiguousarray(Wv.T, dtype=f32)
    WoT = np.ascontiguousarray(Wo.T, dtype=f32)
    bq = np.ascontiguousarray(bq, dtype=f32)
    bk = np.ascontiguousarray(bk, dtype=f32)
    bv = np.ascontiguousarray(bv, dtype=f32)
    bo = np.ascontiguousarray(bo, dtype=f32)
    KTs = [np.ascontiguousarray(K[b].T, dtype=f32) for b in range(B)]
    VTs = [np.ascontiguousarray(V[b].T, dtype=f32) for b in range(B)]
    in_maps = []
    for c in range(NCORES):
        b, half = divmod(c, 2)
        qt = np.ascontiguousarray(Q[b, half * TOK:(half + 1) * TOK, :].T, dtype=f32)
        in_maps.append({
            "qt": qt, "kt": KTs[b], "vt": VTs[b],
            "wqt": WqT, "wkt": WkT, "wvt": WvT, "wot": WoT,
            "bq": bq, "bk": bk, "bv": bv, "bo": bo,
        })
    return in_maps


def _run(V, K, Q, Wv, bv, Wk, bk, Wq, bq, Wo, bo, **run_kwargs):
    nc = _get_nc()
    in_maps = _make_in_maps(V, K, Q, Wv, bv, Wk, bk, Wq, bq, Wo, bo)
    res = run_bass_kernel_spmd(nc, in_maps, core_ids=list(range(NCORES)), **run_kwargs)
    out = np.empty((B, L, DM), np.float32)
    for c, r in enumerate(res.results):
        b, half = divmod(c, 2)
        out[b, half * TOK:(half + 1) * TOK, :] = r["out"]
    return out, res


def kernel(V, K, Q, Wv, bv, Wk, bk, Wq, bq, Wo, bo):
    out, _ = _run(V, K, Q, Wv, bv, Wk, bk, Wq, bq, Wo, bo)
    return out
